# revision 39
# baseline (speedup 1.0000x reference)
"""Trainium2 Bass kernel for the AssociativeLIF problem.

Strategy v12
------------
Data-parallel over batch: 64 batches -> 8 NeuronCores x 8 batches.
Per core, neurons+batches pack into a (128 partitions, NK*BL free) tile
with slot (p, k*BL+b) holding (neuron n = k*128+p, batch b) — k-major,
so each k-slice of the spike tile is a contiguous [128, BL] block; for
the graded inputs cluster_ids = arange(N) % 64 so the cluster id is
p % 64 and the cascade is a constant-weight 128x128 matmul.

The serial chain runs on DVE (the only engine with STT/is_ge and PSUM
access on real HW).  The baseline's per-step free-dim reduce R is
eliminated: the cascade matmuls consume the spike tile's k-slices
directly, accumulating into an R-space psum [128, BL]

    psum_casc[p', b] = sum_k phi.T @ s[:, k-slice]     (16 fp16 matmuls)

which the next IPx reads with a stride-0 broadcast over k.  The x
injection moves off PE into a Pool-built Q' = bsyn*IPx + xs (two
TensorTensor ops — the only compute the Pool engine legally supports is
TT add/mult).  Per step:

  DVE:   IPx  = Q'(t-1) + psum_casc(t-1)     [TT, the psum read]
         vpre = bmem*v + IPx                 [STT]
         s    = (vpre - th) >= q64           [STT, is_ge, f16 out]
  Pool:  D    = bsyn*IPx ; Q' = D + xs(t+1)  [TT mult/add]
         nths = -th*s    ; v  = vpre + nths  [soft reset, TT]
         s64  = 64*s ; q64(t+2) = s64(t+1)+s64(t)   [exact f16 ints]
  PE:    3 warmers (0-weights, wake on vpre and bridge the engine
         into the compare's semaphore window) + 16 cascade matmuls

q64 in SBUF: for q=0 both sides compare against exactly 0.0 so
decisions match the reference bit-for-bit; for q>=1 the blocker is
64/128 > max|vpre|.  Pool compute ops have no ISA wait slots; 1-elem
tensor-copy shims carry every cross-engine wait into the Pool stream.
"""
import math
import sys

import numpy as np

sys.path.insert(0, "/opt/trn_rl_repo")

import os

B, T, N, NCDIM = 64, 512, 1024, 64
T = int(os.environ.get("LIF_T_OVERRIDE", T))   # for scaling experiments
NCORES = 8
BL = B // NCORES          # 8 batches per core
CH = 16                   # timesteps per DMA chunk
NCHUNK = T // CH


def _mk_out_splits(t):
    """<=8 output pieces (one SWDGE queue each), later pieces smaller to
    shrink the post-loop drain tail (the last piece is just 4 steps)."""
    fr = [0, 400, 800, 1060, 1180, 1240, 1265, 1278, 1280]
    s = sorted(set(min(t, v * t // 1280) for v in fr))
    if s[-1] != t:
        s.append(t)
    return s

_graph_cache = {}


def _build_graph(NK, bsyn, bmem, th_imm, th_general):
    """Per-core Bass graph v12."""
    from contextlib import ExitStack

    import concourse.bass as bass
    from concourse import mybir
    from concourse.tile import TileContext

    f32 = mybir.dt.float32
    f16 = mybir.dt.float16
    FREE = BL * NK
    aop = mybir.AluOpType

    nc = bass.Bass()
    x_dram = nc.declare_dram_parameter("x", [NCHUNK, 128, CH * FREE], f32,
                                       isOutput=False)
    # all f32 constants ride in one DMA: [phi3 | ident | th_dev]
    ncst = 256 + FREE
    cst_dram = nc.declare_dram_parameter("cst", [128, ncst], f32,
                                         isOutput=False)
    cstb_dram = nc.declare_dram_parameter("cstb", [128, 384], f16,
                                          isOutput=False)
    out_splits = _mk_out_splits(T)
    npieces = len(out_splits) - 1
    assert out_splits[-1] == T and npieces <= 8
    out_drams = [
        nc.declare_dram_parameter(
            f"out{i}", [128, (out_splits[i + 1] - out_splits[i]) * FREE],
            mybir.dt.float16, isOutput=True)
        for i in range(npieces)
    ]

    with TileContext(nc) as tc, ExitStack() as ctx:
        consts = ctx.enter_context(tc.tile_pool(name="consts", bufs=1))
        state = ctx.enter_context(tc.tile_pool(name="state", bufs=1))
        step2 = ctx.enter_context(tc.tile_pool(name="step2", bufs=2))
        pIC = ctx.enter_context(
            tc.tile_pool(name="pIC", bufs=4, space="PSUM"))

        cst = consts.tile([128, ncst], f32, tag="cst")
        phi3 = cst[:, 0:128]
        ident = cst[:, 128:256]
        thdev = cst[:, 256:256 + FREE]
        cstb = consts.tile([128, 384], f16, tag="cstb")
        phi3b = cstb[:, 0:128]
        phi3r = cstb[:, 128:256]
        zw16 = consts.tile([128, 128], f16, tag="zw16")
        nc.vector.memset(zw16, 0.0)
        zw32 = consts.tile([128, 128], f32, tag="zw32")
        nc.vector.memset(zw32, 0.0)

        # x and s live in static write-once SBUF regions (no WAR/WAW)
        xstat = consts.tile([128, T * FREE], f32, tag="xstat")
        sstat = consts.tile([128, T * FREE], f16, tag="sstat")
        # write-once 1-elem shim stripes (wait carriers for Pool ops)
        shA = consts.tile([128, 2 * T + 2], f16, tag="shA")

        izero = state.tile([128, FREE], f32, tag="izero")
        nc.vector.memset(izero, 0.0)
        z16 = state.tile([128, FREE], f16, tag="z16")
        nc.vector.memset(z16, 0.0)
        c64C = state.tile([128, FREE], f16, tag="c64C")
        nc.gpsimd.memset(c64C, 64.0)
        bsynC = state.tile([128, FREE], f32, tag="bsynC")
        nc.gpsimd.memset(bsynC, bsyn)
        v = state.tile([128, FREE], f32, tag="v")
        nc.gpsimd.memset(v, 0.0)
        nthC = state.tile([128, FREE], f32, tag="nthC")
        thvC = state.tile([128, FREE], f32, tag="thvC")
        if th_general:
            nc.gpsimd.tensor_copy(out=thvC, in_=thdev)
            # nthC = -th per slot: thdev * (-1)  (Pool TT mult)
            none1 = state.tile([128, FREE], f32, tag="none1")
            nc.gpsimd.memset(none1, -1.0)
            nc.gpsimd.tensor_tensor(out=nthC, in0=thdev, in1=none1,
                                    op=aop.mult)
        else:
            nc.gpsimd.memset(thvC, th_imm)
            nc.gpsimd.memset(nthC, -th_imm)

        CW = CH * FREE
        # chunk-0 sliver first: step 0 starts after a 2-step transfer,
        # the consts (needed only by the step-0 cascade) follow
        nc.sync.dma_start(out=cstb, in_=cstb_dram[:, :])
        nc.sync.dma_start(out=xstat[:, 0:2 * FREE],
                          in_=x_dram[0][:, 0:2 * FREE])
        nc.sync.dma_start(out=xstat[:, 2 * FREE:CW],
                          in_=x_dram[0][:, 2 * FREE:CW])
        for c in range(1, NCHUNK):
            nc.sync.dma_start(out=xstat[:, c * CW:(c + 1) * CW],
                              in_=x_dram[c])
        # cst (phi3/ident/th_dev) is only read by the general-th path
        nc.sync.dma_start(out=cst, in_=cst_dram[:, :])

        # boot absorbs: pull cross-engine boot writes into each stream
        pboot = state.tile([128, 1], f32, tag="pboot")
        nc.vector.tensor_copy(out=pboot, in_=izero[:, 0:1])
        nc.gpsimd.tensor_copy(out=shA[:, 2 * T:2 * T + 1], in_=z16[:, 0:1])

        # bootstrap: psum_casc(-1) = 0 (0-weight matmul); Q'(-1) = xs(0)
        pc = pIC.tile([128, BL], f32, tag="pc")
        nc.tensor.matmul(out=pc, lhsT=zw16, rhs=z16[:, 0:BL],
                         start=True, stop=True)
        qprev = xstat[:, 0:FREE]

        q64_cur = None
        s64_prev = None
        deferred_absorbs = []

        for t in range(T):
            ssl = sstat[:, t * FREE:(t + 1) * FREE]

            # DVE: IPx = Q'(t-1) + psum_casc(t-1)[bcast over k]
            pcb = pc[:, :].rearrange("p (o b) -> p o b", o=1) \
                .broadcast_to((128, NK, BL))
            ipx = step2.tile([128, FREE], f32, tag="ipx", bufs=2)
            nc.vector.tensor_tensor(out=ipx, in0=qprev, in1=pcb,
                                    op=aop.add)
            # DVE: vpre = bmem*v + IPx
            vpre = step2.tile([128, FREE], f32, tag="vpre", bufs=2)
            nc.vector.scalar_tensor_tensor(
                out=vpre, in0=(izero if t == 0 else v), scalar=bmem,
                in1=ipx, op0=aop.mult, op1=aop.add)
            # DVE: s = (vpre - th) >= q64
            if th_general:
                nc.vector.tensor_tensor(
                    out=ssl, in0=vpre, in1=(q64_cur if t > 0 else thvC),
                    op=aop.is_ge)
            else:
                nc.vector.scalar_tensor_tensor(
                    out=ssl, in0=vpre, scalar=th_imm,
                    in1=(q64_cur if t > 0 else z16),
                    op0=aop.subtract, op1=aop.is_ge)

            # PE: warmer (wakes on the compare; 0-weights keep the psum
            # exact) + per-k cascade matmuls into R-space psum(t)
            if t + 1 < T:
                # two 0-weight f32 warmers wake on vpre and bridge the
                # PE into the compare's semaphore window, so the casc
                # matmuls start within ~12ns of s landing
                pc = pIC.tile([128, BL], f32, tag="pc")
                for w in range(3):
                    nc.tensor.matmul(out=pc, lhsT=zw32,
                                     rhs=vpre[:, 0:BL],
                                     start=(w == 0), stop=False)
                for k in range(NK):
                    sk = ssl[:, k * BL:(k + 1) * BL]
                    nc.tensor.matmul(out=pc, lhsT=phi3b, rhs=sk,
                                     start=False, stop=False)
                for k in range(NK):
                    sk = ssl[:, k * BL:(k + 1) * BL]
                    nc.tensor.matmul(out=pc, lhsT=phi3r, rhs=sk,
                                     start=False, stop=(k == NK - 1))

            # Pool: shims carry the DVE waits (ipx, s) into the Pool
            # stream; then TT-only helpers (the only legal Pool compute)
            nc.gpsimd.tensor_copy(out=shA[:, 2 * t:2 * t + 1],
                                  in_=ipx[:, 0:1])
            # Q'(t) = bsyn*IPx(t) + xs(t+1)
            if t + 1 < T:
                dq = step2.tile([128, FREE], f32, tag="dq", bufs=2)
                nc.gpsimd.tensor_tensor(out=dq, in0=ipx, in1=bsynC,
                                        op=aop.mult)
                qp = step2.tile([128, FREE], f32, tag="qp", bufs=2)
                nc.gpsimd.tensor_tensor(
                    out=qp, in0=dq,
                    in1=xstat[:, (t + 1) * FREE:(t + 2) * FREE],
                    op=aop.add)
                qprev = qp[:, :]
            nc.gpsimd.tensor_copy(out=shA[:, 2 * t + 1:2 * t + 2],
                                  in_=ssl[:, 0:1])
            # soft reset: v = vpre + (-th)*s
            nths = step2.tile([128, FREE], f32, tag="nths", bufs=2)
            nc.gpsimd.tensor_tensor(out=nths, in0=ssl, in1=nthC,
                                    op=aop.mult)
            nc.gpsimd.tensor_tensor(out=v, in0=vpre, in1=nths,
                                    op=aop.add)
            # next-step refractory bias (exact f16 ints)
            s64 = step2.tile([128, FREE], f16, tag="s64", bufs=3)
            with nc.allow_low_precision(
                    reason="64*s in {0,64} is exact in fp16"):
                nc.gpsimd.tensor_tensor(out=s64, in0=ssl, in1=c64C,
                                        op=aop.mult)
            if t + 1 < T:
                if t == 0:
                    q64_next = s64[:, :]
                else:
                    q64n = step2.tile([128, FREE], f16, tag="q64",
                                      bufs=2)
                    with nc.allow_low_precision(
                            reason="64*(s+s') in {0,64,128} exact"):
                        nc.gpsimd.tensor_tensor(
                            out=q64n, in0=s64, in1=s64_prev, op=aop.add)
                    q64_next = q64n[:, :]
                if th_general:
                    thq = step2.tile([128, FREE], f32, tag="thq",
                                     bufs=2)
                    nc.gpsimd.tensor_tensor(
                        out=thq, in0=q64_next, in1=thvC, op=aop.add)
                    q64_next = thq[:, :]
                q64_cur = q64_next
            s64_prev = s64[:, :]

            t_end = t + 1
            if t_end in out_splits:
                i = out_splits.index(t_end) - 1
                t0 = out_splits[i]
                nc.sync.dma_start(
                    out=out_drams[i][:, :],
                    in_=sstat[:, t0 * FREE:t_end * FREE])
                deferred_absorbs.append(t0)

        # final piece
        i = npieces - 1
        t0 = out_splits[i]
        nc.sync.dma_start(out=out_drams[i][:, :],
                          in_=sstat[:, t0 * FREE:T * FREE])
        deferred_absorbs.append(t0)

    _prune_drain_waits(nc)
    _split_multi_waits(nc)
    return nc


def _split_multi_waits(nc):
    """The compute-instruction ISA structs hold a single sync wait.  Any
    instruction left with more is split: the extra waits move to an
    InstEventSemaphore inserted just before it on the same engine
    (EventSemaphore holds up to 2 waits per instance)."""
    from concourse import mybir
    nsh = 0
    for fn in nc.m.functions:
        for blk in fn.blocks:
            out = []
            changed = False
            for inst in blk.instructions:
                si = getattr(inst, "sync_info", None)
                if (si is not None and si.on_wait and len(si.on_wait) > 1
                        and type(inst).__name__ not in (
                            "InstDrain", "InstEventSemaphore")):
                    waits = list(si.on_wait)
                    extra, keep = waits[:-1], waits[-1:]
                    for i in range(0, len(extra), 2):
                        nsh += 1
                        ev = mybir.InstEventSemaphore(
                            name=f"{inst.name}-evw{i}",
                            engine=inst.engine,
                            ins=[], outs=[],
                            sync_info=mybir.SyncInfo(
                                on_wait=extra[i:i + 2], on_update=[]),
                        )
                        out.append(ev)
                        nc.inst_map[ev.name] = ev
                    si.on_wait = keep
                    changed = True
                out.append(inst)
            if changed:
                blk.instructions = out
    return nsh


def _prune_drain_waits(nc):
    """The kernel-tail drain waits on every proc — over the SP wait-slot
    limit.  Drop the DMAHW (x-input) waits: every input DMA has a
    compute consumer ordered after it, so that wait already implies
    their completion. Output (DMASW), DVE and PE waits are kept."""
    for inst in nc.inst_map.values():
        si = getattr(inst, "sync_info", None)
        if type(inst).__name__ != "InstDrain" or not si or not si.on_wait:
            continue
        if len(si.on_wait) > 1:
            kept = [w for w in si.on_wait if w.ant_name.startswith("Pool")]
            si.on_wait = kept[:1] if kept else si.on_wait[:1]


def _np_sigmoid32(x):
    # float64 sigmoid then cast — matches jax f32 sigmoid to <=1 ulp
    return np.float32(1.0 / (1.0 + math.exp(-float(x))))


def _numpy_fallback(x, th, bmem, bsyn, W, gain, cids, div):
    """Exact float32 mirror of the reference dynamics, used when the
    cluster packing does not fit the device kernel's static-SBUF layout.
    Slow but correct for any geometry."""
    B_, T_, N_ = x.shape
    nc_ = gain.shape[0]
    M = np.zeros((N_, nc_), np.float32)
    M[np.arange(N_), cids] = 1.0
    nbmem = np.float32(1.0) - bmem
    v = np.zeros((B_, N_), np.float32)
    i_syn = np.zeros((B_, N_), np.float32)
    refrac = np.zeros((B_, N_), np.int32)
    out = np.zeros((B_, T_, N_), np.float32)
    for t in range(T_):
        i_syn = bsyn * i_syn + x[:, t, :]
        v = bmem * v + nbmem * i_syn
        s = ((v >= th) & (refrac == 0)).astype(np.float32)
        cf = (s @ M) / np.float32(div)
        casc = ((cf @ W.T) * gain)[:, cids]
        i_syn = i_syn + casc
        v = v - s * th
        refrac = np.where(s > 0, 2, np.maximum(refrac - 1, 0))
        out[:, t, :] = s
    return out


def prepare(x, v_threshold_raw, beta_mem_raw, beta_syn_raw,
            neighbor_weights, cluster_gain, cluster_ids):
    """Host-side preprocessing: returns (nc, in_maps, postprocess) where
    postprocess(results_list) -> full (B, T, N) float32 output."""
    x = np.asarray(x, np.float32)
    Bb, Tt, Nn = x.shape
    assert (Bb, Tt, Nn) == (B, T, N)
    ncdim = np.asarray(cluster_gain).shape[0]
    div = max(Nn // ncdim, 1)
    th = np.clip(np.asarray(v_threshold_raw, np.float32),
                 np.float32(0.05), np.float32(0.5))
    bmem = np.float32(np.clip(_np_sigmoid32(beta_mem_raw), np.float32(0.8),
                              np.float32(0.98)))
    bsyn = _np_sigmoid32(beta_syn_raw)
    W = (1.0 / (1.0 + np.exp(-np.asarray(neighbor_weights,
                                         np.float64)))).astype(np.float32)
    gain = np.asarray(cluster_gain, np.float32)
    cids = np.asarray(cluster_ids)
    nbmem = np.float32(1.0 - np.float64(bmem))

    arange_case = np.array_equal(cids, np.arange(Nn) % ncdim)
    if arange_case and Nn % 128 == 0:
        NK = Nn // 128
        slot_of_n = (np.arange(Nn) % 128) * NK + (np.arange(Nn) // 128)
        # slot s=(p*NK+k) holds n = k*128+p
        p_idx = np.arange(128 * NK) // NK
        k_idx = np.arange(128 * NK) % NK
        th_slots = th[k_idx * 128 + p_idx]
        valid = np.ones(128 * NK, bool)
    else:
        counts = np.bincount(cids, minlength=ncdim)
        NK = max(1, int(math.ceil(counts.max() / (128 // ncdim))))
        nslot = 128 * NK
        slot_of_n = np.empty(Nn, np.int64)
        fill = np.zeros(ncdim, np.int64)
        per_res = 128 // ncdim  # partitions per residue (2 for nc=64)
        for n in range(Nn):
            c = int(cids[n])
            j = fill[c]
            fill[c] = j + 1
            p = c + ncdim * (j % per_res)
            k = j // per_res
            slot_of_n[n] = p * NK + k
        n_of_slot = np.full(nslot, -1, np.int64)
        n_of_slot[slot_of_n] = np.arange(Nn)
        valid = n_of_slot >= 0
        th_slots = np.full(nslot, 1e9, np.float32)
        th_slots[slot_of_n] = th

    FREE = BL * NK
    if NK > 8:
        return None, (x, th, bmem, bsyn, W, gain, cids, div), None
    th_uniform = bool(np.all(th == th[0])) and bool(valid.all())
    th_imm = float(th[0]) if th_uniform else 0.0

    # Phi[pp, p] = bsyn*nbmem/div * gain[p%nc] * W[p%nc, pp%nc]
    # (bsyn folded in: the cascade joins i_syn one decay step before its
    #  first use in IPx)
    r = np.arange(128) % ncdim
    A = gain[r][:, None] * W[np.ix_(r, r)]          # A[p, pp]
    phi3 = ((np.float32(bsyn) * nbmem / np.float32(div)) * A.T
            ).astype(np.float32)

    # th tile [128, FREE] in k-major layout: th_dev[p, k*BL+b]
    th_dev = np.ascontiguousarray(
        np.broadcast_to(th_slots.reshape(128, NK, 1),
                        (128, NK, BL)).reshape(128, FREE))
    eye = np.eye(128, dtype=np.float32)
    cst = np.ascontiguousarray(np.concatenate(
        [phi3, eye, th_dev], axis=1))
    phi3h = phi3.astype(np.float16)
    phi3r = (phi3 - phi3h.astype(np.float32)).astype(np.float16)
    eye64h = (64.0 * eye).astype(np.float16)
    cstb = np.ascontiguousarray(
        np.concatenate([phi3h, phi3r, eye64h], axis=1))

    # x -> slots -> device layout per core.  Device free layout is
    # k-major: column (ch, k, b); xs pre-scaled by (1-bmem).
    xs = x * nbmem
    in_maps = []
    for ci in range(NCORES):
        xc = xs[ci * BL:(ci + 1) * BL]              # (BL, T, N)
        xslot = np.zeros((BL, Tt, 128 * NK), np.float32)
        xslot[:, :, slot_of_n] = xc
        xdev = np.ascontiguousarray(
            xslot.reshape(BL, NCHUNK, CH, 128, NK)
                 .transpose(1, 3, 2, 4, 0)
                 .reshape(NCHUNK, 128, CH * FREE))
        in_maps.append({"x": xdev, "cst": cst, "cstb": cstb})

    key = (NK, float(bsyn), float(bmem), th_imm, not th_uniform)
    if key not in _graph_cache:
        _graph_cache[key] = _build_graph(NK, float(bsyn), float(bmem),
                                         th_imm, not th_uniform)
    nc = _graph_cache[key]

    npieces = len(_mk_out_splits(T)) - 1

    def postprocess(results):
        out = np.empty((B, Tt, Nn), np.float32)
        for ci in range(NCORES):
            o = np.concatenate(
                [np.asarray(results[ci][f"out{i}"]) for i in range(npieces)],
                axis=1).astype(np.float32)          # (128, T*FREE)
            oslot = (o.reshape(128, Tt, NK, BL)
                      .transpose(3, 1, 0, 2)
                      .reshape(BL, Tt, 128 * NK))
            out[ci * BL:(ci + 1) * BL] = oslot[:, :, slot_of_n]
        return out

    return nc, in_maps, postprocess


def kernel(x, v_threshold_raw, beta_mem_raw, beta_syn_raw,
           neighbor_weights, cluster_gain, cluster_ids):
    from concourse.bass_utils import run_bass_kernel_spmd

    nc, in_maps, postprocess = prepare(
        x, v_threshold_raw, beta_mem_raw, beta_syn_raw,
        neighbor_weights, cluster_gain, cluster_ids)
    if nc is None:
        return _numpy_fallback(*in_maps)
    res = run_bass_kernel_spmd(nc, in_maps, core_ids=list(range(NCORES)))
    return postprocess(res.results)
